# revision 24
# baseline (speedup 1.0000x reference)
"""Trainium2 Bass kernel for nn_Block_2302102471059 (ragged_sequence).

Per-NeuronCore pipeline (8-way shard by GRU group ownership; each core owns
16 of the 128 sequence groups = 8192 slots, slot s = l*16 + g_local):

  - Graph aggregation (segment-mean over in-edges): in_feats is host-cast to
    f16 and split into 3 DRAM tables of <32768 rows (dma_gather indices are
    int16, and the SWDGE descriptor ring caps one gather at 1024 rows).
    Edges are host-bucketed by (dst-tile, table); one bulk dma_gather per
    (tile, table) stages rows into SBUF, then one-hot matmuls scatter-reduce
    them into PSUM. The 1/deg scaling is folded into the layout transpose
    (matmul against diag(invdeg) instead of identity).
  - conv/ff1/gi run in transposed layout (features on partitions). All
    weights arrive pre-transposed/pre-cast f16 from the host.
  - The GRU (L=512 serial steps, batch 16) is the critical path: ~2.6us per
    step of cross-engine chain (PE gh-matmuls -> sigmoid(r) -> r*hn -> +gi_n
    -> tanh -> blend). r-gates get their own PSUM tile so sigmoid(r) fires
    after only 16 matmuls; z/n share a second tile with a single open
    start=True group (two concurrently-open start groups in one PSUM tile
    lose one preload on hardware).
  - All other work (gathers, one-hot, dense matmuls, evictions, ff2 +
    transpose + f16 store) is emitted interleaved into the GRU step slots as
    weighted work items, filling the chain's engine idle gaps.

Timeline-sim/HW exec ~1.48ms (baseline 2.95ms); rel err 9.6e-4 (= exact
f16-pipeline model error; tolerance 2e-2).
"""

import os
import sys

import numpy as np

sys.path.insert(0, "/opt/trn_rl_repo")

from contextlib import ExitStack

import concourse.bacc as bacc
import concourse.bass as bass
import concourse.tile as tile
from concourse import mybir
from concourse.bass_utils import run_bass_kernel_spmd
from concourse.masks import make_identity

N, D, E, G, L = 65536, 512, 1048576, 128, 512
NCORES = 8
GP = G // NCORES          # 16 groups per core
S = GP * L                # 8192 slots per core
NT = S // 128             # 64 dst-tiles of 128 slots
LCH = 32                  # l-steps per stream chunk
NCH = L // LCH            # 16 chunks
F16 = mybir.dt.float16
F32 = mybir.dt.float32
I16 = mybir.dt.int16
I32 = mybir.dt.int32

# 3 gather tables (int16 index limit 32767)
TBASE = [0, 21846, 43691, 65536]
NTAB = 3

LAST_RESULT = None
LAST_NC = None


def _build(meta):
    """meta: dict with
      nblk[t][j]      blocks (128 rows) per (tile, table)
      tsizes[j]       rows per feats table
    """
    nblk = meta["nblk"]
    tsizes = meta["tsizes"]
    nblk_t = [sum(nblk[t]) for t in range(NT)]       # blocks per tile
    maxnblk = max(nblk_t)
    NBLKSUM = sum(nblk_t)                            # total ldst block-cols
    IDXW = 8 * NBLKSUM                               # idx cols (int16)
    # flat offsets
    blk_off = np.zeros((NT, NTAB), dtype=int)
    idx_off = np.zeros((NT, NTAB), dtype=int)
    acc = 0
    for t in range(NT):
        for j in range(NTAB):
            blk_off[t][j] = acc
            idx_off[t][j] = 8 * acc
            acc += nblk[t][j]

    nc = bacc.Bacc("TRN2", target_bir_lowering=False, debug=False)

    # ---- DRAM I/O ----
    feats = [nc.dram_tensor(f"feats{j}", [tsizes[j], D], F16, kind="ExternalInput")
             for j in range(NTAB)]
    d_wconv = nc.dram_tensor("h_wconv", [128, 2048], F16, kind="ExternalInput")
    d_wff1 = nc.dram_tensor("h_wff1", [128, 2048], F16, kind="ExternalInput")
    d_wff2 = nc.dram_tensor("h_wff2", [128, 2048], F16, kind="ExternalInput")
    d_wihT = nc.dram_tensor("h_wihT", [128, 6144], F16, kind="ExternalInput")
    d_whhT = nc.dram_tensor("h_whhT", [128, 6144], F16, kind="ExternalInput")
    d_bconv = nc.dram_tensor("h_bconv", [128, 4], F32, kind="ExternalInput")
    d_bff1 = nc.dram_tensor("h_bff1", [128, 4], F32, kind="ExternalInput")
    d_bff2 = nc.dram_tensor("h_bff2", [128, 4], F32, kind="ExternalInput")
    d_bsum = nc.dram_tensor("h_bsum", [128, 12], F32, kind="ExternalInput")
    d_zbn = nc.dram_tensor("h_zbn", [128, 128], F16, kind="ExternalInput")
    d_invdeg = nc.dram_tensor("h_invdeg", [128, NT], F32, kind="ExternalInput")
    idx2d = nc.dram_tensor("idx2d", [128, IDXW], I16, kind="ExternalInput")
    ldst2d = nc.dram_tensor("ldst2d", [128, NBLKSUM], F32, kind="ExternalInput")
    out = nc.dram_tensor("out", [S, D], F16, kind="ExternalOutput")
    DBG = bool(int(os.environ.get("KDBG", "0")))
    if DBG:
        dbg_xt = nc.dram_tensor("dbg_xt", [128, 2048], F16, kind="ExternalOutput")
        dbg_ct = nc.dram_tensor("dbg_ct", [128, 2048], F16, kind="ExternalOutput")
        dbg_mt = nc.dram_tensor("dbg_mt", [128, 2048], F16, kind="ExternalOutput")
        dbg_git = nc.dram_tensor("dbg_git", [128, 6144], F16, kind="ExternalOutput")
        dbg_ring = nc.dram_tensor("dbg_ring", [128, 4096], F16, kind="ExternalOutput")
        dbg_g0 = nc.dram_tensor("dbg_g0", [128, 5 * 64], F16, kind="ExternalOutput")

    with tile.TileContext(nc) as tc, ExitStack() as ctx:
        wpool = ctx.enter_context(tc.tile_pool(name="w", bufs=1))
        tmp = ctx.enter_context(tc.tile_pool(name="tmp", bufs=2))
        stage = ctx.enter_context(tc.tile_pool(name="stage", bufs=3))
        ohp = ctx.enter_context(tc.tile_pool(name="oh", bufs=2))
        agghp = ctx.enter_context(tc.tile_pool(name="aggh", bufs=2))
        diagp = ctx.enter_context(tc.tile_pool(name="diag", bufs=2))
        xtp = ctx.enter_context(tc.tile_pool(name="xt", bufs=2))
        ctp = ctx.enter_context(tc.tile_pool(name="ct", bufs=2))
        mtp = ctx.enter_context(tc.tile_pool(name="mt", bufs=2))
        gip = ctx.enter_context(tc.tile_pool(name="gi", bufs=2))
        grup = ctx.enter_context(tc.tile_pool(name="gru", bufs=2))
        outp = ctx.enter_context(tc.tile_pool(name="outw", bufs=2))
        ps_a = ctx.enter_context(tc.tile_pool(name="psa", bufs=1, space="PSUM"))
        ps_mm = ctx.enter_context(tc.tile_pool(name="psmm", bufs=2, space="PSUM"))
        ps_gru = ctx.enter_context(tc.tile_pool(name="psgru", bufs=2, space="PSUM"))
        ps_tr = ctx.enter_context(tc.tile_pool(name="pstr", bufs=1, space="PSUM"))

        # ---- constants / meta / weights (host-prepped layouts) ----
        ident = wpool.tile([128, 128], F16, tag="ident")
        make_identity(nc, ident[:])
        iotaf = wpool.tile([128, 128], F32, tag="iotaf")
        iotai = tmp.tile([128, 128], I32, tag="ioi")
        nc.gpsimd.iota(iotai[:], pattern=[[1, 128]], base=0, channel_multiplier=0)
        nc.vector.tensor_copy(iotaf[:], iotai[:])
        zero64 = wpool.tile([128, 64], F16, tag="zero64")
        nc.vector.memset(zero64[:], 0.0)

        idx_sb = wpool.tile([128, IDXW], I16, tag="idxsb")
        nc.sync.dma_start(out=idx_sb[:], in_=idx2d[:, :])
        ldst_f = wpool.tile([128, NBLKSUM], F32, tag="ldstf")
        nc.sync.dma_start(out=ldst_f[:], in_=ldst2d[:, :])
        invdeg = wpool.tile([128, NT], F32, tag="invdeg")
        nc.sync.dma_start(out=invdeg[:], in_=d_invdeg[:, :])

        def loadw(dram, cols, dt, tag):
            t = wpool.tile([128, cols], dt, tag=tag)
            nc.sync.dma_start(out=t[:], in_=dram[:, :])
            return t

        wconv = loadw(d_wconv, 2048, F16, "wconv")
        wff1 = loadw(d_wff1, 2048, F16, "wff1")
        wff2 = loadw(d_wff2, 2048, F16, "wff2")
        wihT = loadw(d_wihT, 6144, F16, "wihT")
        whhT = loadw(d_whhT, 6144, F16, "whhT")
        bconv = loadw(d_bconv, 4, F32, "bconv")
        bff1 = loadw(d_bff1, 4, F32, "bff1")
        bff2 = loadw(d_bff2, 4, F32, "bff2")
        bsum = loadw(d_bsum, 12, F32, "bsum")
        zbn = loadw(d_zbn, 128, F16, "zbn")

        # GRU hidden ring buffer: 64 l-slots x [4 d-chunks x 16 groups]
        ring = wpool.tile([128, 64 * 64], F16, tag="ring")

        def gather_tile(t):
            """Issue the 3 dma_gathers for tile t into a staging tile;
            returns (st, nb) with blocks of the 3 tables back to back."""
            nb = nblk_t[t]
            st = stage.tile([128, maxnblk * D], F16, tag="st")
            b0 = 0
            for j in range(NTAB):
                nbj = nblk[t][j]
                if nbj == 0:
                    continue
                nidx = 128 * nbj
                nc.gpsimd.dma_gather(
                    out_ap=st[:, b0 * D:(b0 + nbj) * D].rearrange(
                        "p (b c) -> p b c", c=D),
                    in_ap=feats[j][:, :],
                    idxs_ap=idx_sb[:, idx_off[t][j]: idx_off[t][j] + 8 * nbj],
                    num_idxs=nidx,
                    num_idxs_reg=nidx,
                    elem_size=D,
                )
                b0 += nbj
            return st, nb

        def gru_step(t_step, gi_t, gi_base):
            """One GRU step. Critical chain: r-mms -> sigmoid(r) -> rhn ->
            tg -> tanh -> mm_ -> hadd. z-sigmoid, omz, zh run off-chain."""
            if t_step == 0:
                h_prev = zero64[:]
            else:
                o = ((t_step - 1) % 64) * 64
                h_prev = ring[:, o:o + 64]
            psr = ps_gru.tile([128, 64], F32, space="PSUM", tag="psr")
            pszn = ps_gru.tile([128, 128], F32, space="PSUM", tag="pszn")
            # gi/bias preloads (independent of h). NOTE: exactly ONE
            # start=True group may be open per PSUM tile: a second open
            # start into another column range of the same tile gets dropped
            # on hardware (z-gate bug found 2026-08). zbn = [zeros | b_hh_n].
            nc.tensor.matmul(psr[:], ident[:], gi_t[:, gi_base:gi_base + 64],
                             start=True, stop=False)
            nc.tensor.matmul(pszn[:], ident[:], zbn[:],
                             start=True, stop=False)
            nc.tensor.matmul(pszn[:, 0:64], ident[:],
                             gi_t[:, gi_base + 64:gi_base + 128],
                             start=False, stop=False)
            # r gates (j 0..3) into psr -> sigmoid(r) fires earliest
            for j in range(4):
                for c in range(4):
                    nc.tensor.matmul(
                        psr[:, j * 16:(j + 1) * 16],
                        whhT[:, c * 1536 + j * 128: c * 1536 + (j + 1) * 128],
                        h_prev[:, c * 16:(c + 1) * 16],
                        start=False, stop=(c == 3))
            for j in range(4, 8):
                for c in range(4):
                    nc.tensor.matmul(
                        pszn[:, (j - 4) * 16:(j - 3) * 16],
                        whhT[:, c * 1536 + j * 128: c * 1536 + (j + 1) * 128],
                        h_prev[:, c * 16:(c + 1) * 16],
                        start=False, stop=(c == 3))
            for j in range(8, 12):
                for c in range(4):
                    nc.tensor.matmul(
                        pszn[:, 64 + (j - 8) * 16:64 + (j - 7) * 16],
                        whhT[:, c * 1536 + j * 128: c * 1536 + (j + 1) * 128],
                        h_prev[:, c * 16:(c + 1) * 16],
                        start=False, stop=(c == 3))
            sigr = grup.tile([128, 64], F16, tag="sigr")
            nc.scalar.activation(sigr[:], psr[:],
                                 mybir.ActivationFunctionType.Sigmoid)
            sigz = grup.tile([128, 64], F16, tag="sigz")
            nc.scalar.activation(sigz[:], pszn[:, 0:64],
                                 mybir.ActivationFunctionType.Sigmoid)
            rhn = grup.tile([128, 64], F16, tag="rhn")
            nc.vector.tensor_mul(rhn[:], sigr[:], pszn[:, 64:128])
            tg = grup.tile([128, 64], F16, tag="tg")
            nc.vector.tensor_add(tg[:], rhn[:], gi_t[:, gi_base + 128:gi_base + 192])
            n_t = grup.tile([128, 64], F16, tag="nt")
            nc.scalar.activation(n_t[:], tg[:], mybir.ActivationFunctionType.Tanh)
            omz = grup.tile([128, 64], F16, tag="omz")
            nc.vector.tensor_scalar(out=omz[:], in0=sigz[:],
                                    scalar1=-1.0, scalar2=1.0,
                                    op0=mybir.AluOpType.mult,
                                    op1=mybir.AluOpType.add)
            zh = grup.tile([128, 64], F16, tag="zh")
            nc.vector.tensor_mul(zh[:], sigz[:], h_prev)
            mm_ = grup.tile([128, 64], F16, tag="mm")
            nc.vector.tensor_mul(mm_[:], omz[:], n_t[:])
            hslot = ring[:, (t_step % 64) * 64:(t_step % 64) * 64 + 64]
            nc.vector.tensor_add(hslot, mm_[:], zh[:])
            if DBG and t_step == 0:
                pzdump = grup.tile([128, 64], F16, tag="pzdump")
                nc.scalar.activation(pzdump[:], pszn[:, 0:64],
                                     mybir.ActivationFunctionType.Copy)
                nc.sync.dma_start(out=dbg_g0[:, 256:320], in_=pzdump[:])
                nc.sync.dma_start(out=dbg_g0[:, 0:64], in_=sigr[:])
                nc.sync.dma_start(out=dbg_g0[:, 64:128], in_=sigz[:])
                nc.sync.dma_start(out=dbg_g0[:, 128:192], in_=n_t[:])
                nc.sync.dma_start(out=dbg_g0[:, 192:256], in_=omz[:])

        def ff2_items(k):
            """Items computing ff2 for chunk k's slots from the ring."""
            l0 = (LCH * k) % 64
            rr = ring[:].rearrange("p (l q) -> p l q", q=64)
            state = {}

            def mk_m(m):
                def it():
                    if DBG and k == 0 and m == 0:
                        nc.sync.dma_start(out=dbg_ring[:, :], in_=ring[:])
                    if "ot" not in state:
                        ot_new = outp.tile([128, 4 * 512], F16, tag="ot")
                        state["ot"] = ot_new
                    ot = state["ot"]
                    ps = ps_mm.tile([128, 512], F32, space="PSUM", tag="ps512")
                    for c in range(4):
                        nc.tensor.matmul(
                            ps[:], wff2[:, c * 512 + m * 128: c * 512 + (m + 1) * 128],
                            rr[:, l0:l0 + LCH, c * 16:(c + 1) * 16],
                            start=(c == 0), stop=(c == 3))
                    nc.scalar.activation(ot[:, m * 512:(m + 1) * 512], ps[:],
                                         mybir.ActivationFunctionType.Identity,
                                         bias=bff2[:, m:m + 1])
                return it

            def mk_q(q):
                def it():
                    ot = state["ot"]
                    orow = outp.tile([128, 512], F16, tag="orow")
                    pt = ps_tr.tile([128, 512], F32, space="PSUM", tag="ptr32")
                    for m in range(4):
                        nc.tensor.matmul(
                            pt[:, m * 128:(m + 1) * 128],
                            ot[:, m * 512 + q * 128: m * 512 + (q + 1) * 128],
                            ident[:], start=True, stop=True)
                    nc.vector.tensor_copy(orow[:, 0:256], pt[:, 0:256])
                    nc.vector.tensor_copy(orow[:, 256:512], pt[:, 256:512])
                    nc.sync.dma_start(
                        out=out[k * 512 + q * 128: k * 512 + (q + 1) * 128, :],
                        in_=orow[:])
                return it

            return ([(900, mk_m(m)) for m in range(4)] +
                    [(400, mk_q(q)) for q in range(4)])

        def chunk_dense_items(k):
            """Items for chunk k: gathers, one-hot agg, conv, ff1, gi."""
            state = {}

            def mk_gather(tt):
                t = 4 * k + tt

                def it():
                    st, nb = gather_tile(t)
                    state[tt] = st
                return it

            def mk_oh(tt, g0):
                t = 4 * k + tt

                def it():
                    st = state[tt]
                    nb = nblk_t[t]
                    if g0 >= nb:
                        return
                    gn = min(3, nb - g0)
                    if ("psa", tt) not in state:
                        psa_new = ps_a.tile([128, 512], F32,
                                            space="PSUM", tag="psa")
                        state[("psa", tt)] = psa_new
                    psa = state[("psa", tt)]
                    oh = ohp.tile([128, 3 * 128], F16, tag="oh")
                    src = ldst_f[:, blk_off[t][0] + g0: blk_off[t][0] + g0 + gn]
                    src3 = bass.AP(src.tensor, src.offset,
                                   [src.ap[0], src.ap[1], [0, 128]])
                    io3 = bass.AP(iotaf[:].tensor, iotaf[:].offset,
                                  [iotaf[:].ap[0], [0, gn], iotaf[:].ap[1]])
                    oh3 = oh[:, 0:gn * 128].rearrange("p (q m) -> p q m", m=128)
                    nc.vector.tensor_tensor(out=oh3, in0=src3, in1=io3,
                                            op=mybir.AluOpType.is_equal)
                    for i in range(gn):
                        b = g0 + i
                        nc.tensor.matmul(
                            psa[:], oh[:, i * 128:(i + 1) * 128],
                            st[:, b * D:(b + 1) * D],
                            start=(b == 0), stop=(b == nb - 1))
                return it

            def mk_aggfin(tt):
                t = 4 * k + tt

                def it():
                    psa = state.pop(("psa", tt))
                    if "xt" not in state:
                        xt_new = xtp.tile([128, 4 * 512], F16, tag="xt")
                        state["xt"] = xt_new
                    xt = state["xt"]
                    aggh = agghp.tile([128, 512], F16, tag="aggh")
                    nc.scalar.activation(aggh[:], psa[:],
                                         mybir.ActivationFunctionType.Copy)
                    diag = diagp.tile([128, 128], F16, tag="diag")
                    nc.vector.tensor_scalar(out=diag[:], in0=ident[:],
                                            scalar1=invdeg[:, t:t + 1],
                                            scalar2=None,
                                            op0=mybir.AluOpType.mult)
                    pt = ps_tr.tile([128, 512], F32, space="PSUM", tag="ptr32")
                    for c in range(4):
                        nc.tensor.matmul(pt[:, c * 128:(c + 1) * 128],
                                         aggh[:, c * 128:(c + 1) * 128], diag[:],
                                         start=True, stop=True)
                    xv = xt[:].rearrange("p (c m) -> p c m", m=512)
                    nc.scalar.activation(xv[:, :, tt * 128:(tt + 1) * 128],
                                         pt[:].rearrange("p (c m) -> p c m", m=128),
                                         mybir.ActivationFunctionType.Copy)
                return it

            def mk_conv(m):
                def it():
                    xt = state["xt"]
                    if "ct" not in state:
                        ct_new = ctp.tile([128, 4 * 512], F16, tag="ct")
                        state["ct"] = ct_new
                    ct = state["ct"]
                    ps = ps_mm.tile([128, 512], F32, space="PSUM", tag="ps512")
                    for c in range(4):
                        nc.tensor.matmul(
                            ps[:],
                            wconv[:, c * 512 + m * 128: c * 512 + (m + 1) * 128],
                            xt[:, c * 512:(c + 1) * 512],
                            start=(c == 0), stop=(c == 3))
                    nc.scalar.activation(ct[:, m * 512:(m + 1) * 512], ps[:],
                                         mybir.ActivationFunctionType.Identity,
                                         bias=bconv[:, m:m + 1])
                return it

            def mk_ff1(m):
                def it():
                    ct = state["ct"]
                    if "mt" not in state:
                        mt_new = mtp.tile([128, 4 * 512], F16, tag="mt")
                        state["mt"] = mt_new
                    mt = state["mt"]
                    ps = ps_mm.tile([128, 512], F32, space="PSUM", tag="ps512")
                    for c in range(4):
                        nc.tensor.matmul(
                            ps[:],
                            wff1[:, c * 512 + m * 128: c * 512 + (m + 1) * 128],
                            ct[:, c * 512:(c + 1) * 512],
                            start=(c == 0), stop=(c == 3))
                    nc.scalar.activation(mt[:, m * 512:(m + 1) * 512], ps[:],
                                         mybir.ActivationFunctionType.Relu,
                                         bias=bff1[:, m:m + 1])
                return it

            def mk_gi(j):
                def it():
                    mt = state["mt"]
                    if "git" not in state:
                        git_new = gip.tile([128, LCH * 192], F16, tag="git")
                        state["git"] = git_new
                        gi_tiles[k] = git_new
                    git = state["git"]
                    gir = git[:].rearrange("p (l j b) -> p l j b", j=12, b=16)
                    ps = ps_mm.tile([128, 512], F32, space="PSUM", tag="ps512")
                    for c in range(4):
                        nc.tensor.matmul(
                            ps[:],
                            wihT[:, c * 1536 + j * 128: c * 1536 + (j + 1) * 128],
                            mt[:, c * 512:(c + 1) * 512],
                            start=(c == 0), stop=(c == 3))
                    nc.scalar.activation(
                        gir[:, :, j, :],
                        ps[:].rearrange("p (l b) -> p l b", b=16),
                        mybir.ActivationFunctionType.Identity,
                        bias=bsum[:, j:j + 1])
                    if DBG and k == 0 and j == 11:
                        nc.sync.dma_start(out=dbg_xt[:, :], in_=state["xt"][:])
                        nc.sync.dma_start(out=dbg_ct[:, :], in_=state["ct"][:])
                        nc.sync.dma_start(out=dbg_mt[:, :], in_=state["mt"][:])
                        nc.sync.dma_start(out=dbg_git[:, :], in_=git[:])
                return it

            items = []
            items.append((100, mk_gather(0)))
            items.append((100, mk_gather(1)))
            items.append((100, mk_gather(2)))
            for g0 in range(0, maxnblk, 3):
                items.append((700, mk_oh(0, g0)))
            items.append((500, mk_aggfin(0)))
            items.append((100, mk_gather(3)))
            for g0 in range(0, maxnblk, 3):
                items.append((700, mk_oh(1, g0)))
            items.append((500, mk_aggfin(1)))
            for g0 in range(0, maxnblk, 3):
                items.append((700, mk_oh(2, g0)))
            items.append((500, mk_aggfin(2)))
            for g0 in range(0, maxnblk, 3):
                items.append((700, mk_oh(3, g0)))
            items.append((500, mk_aggfin(3)))
            for m in range(4):
                items.append((900, mk_conv(m)))
            for m in range(4):
                items.append((900, mk_ff1(m)))
            for j in range(12):
                items.append((600, mk_gi(j)))
            return items

        # ================= software-pipelined main loop =================
        from collections import deque
        gi_tiles = {}
        pending = deque()
        pending.extend(chunk_dense_items(0))
        while pending:
            pending.popleft()[1]()
        for k in range(NCH):
            if k + 1 < NCH:
                pending.extend(chunk_dense_items(k + 1))
            if k >= 1:
                pending.extend(ff2_items(k - 1))
            total_w = sum(w for w, _ in pending)
            budget_per_slot = max(400, total_w // (LCH - 4))
            git = gi_tiles.pop(k)
            for li in range(LCH):
                gru_step(k * LCH + li, git, li * 192)
                if li < 2:
                    continue
                spent = 0
                while pending and spent < budget_per_slot:
                    w, it = pending.popleft()
                    it()
                    spent += w
            while pending:
                pending.popleft()[1]()
        for _, it in ff2_items(NCH - 1):
            it()

    nc.compile()
    return nc


def _host_prep(inputs):
    """Bucket edges by (dst tile, src table) per core; build per-core arrays."""
    seq_ids = np.asarray(inputs["seq_ids"]).astype(np.int64)
    edge_src = np.asarray(inputs["edge_src"]).astype(np.int64)
    edge_dst = np.asarray(inputs["edge_dst"]).astype(np.int64)

    counts = np.bincount(edge_dst, minlength=N)
    order = np.argsort(edge_dst, kind="stable")
    src_sorted = edge_src[order].astype(np.int32)
    rowptr = np.zeros(N + 1, dtype=np.int64)
    np.cumsum(counts, out=rowptr[1:])

    tb = np.asarray(TBASE)

    per_core_raw = []
    cnt_blocks = np.zeros((NCORES, NT, NTAB), dtype=np.int64)
    for c in range(NCORES):
        sn = seq_ids[c * GP:(c + 1) * GP, :].T.reshape(-1)  # [S] slot->node
        deg = counts[sn]
        starts = rowptr[sn]
        tot = int(deg.sum())
        # segmented arange gather of all edges in slot order
        csum = np.cumsum(deg) - deg
        seg = np.arange(tot, dtype=np.int64) - np.repeat(csum, deg)
        esrc = src_sorted[np.repeat(starts, deg) + seg]
        slot_ids = np.repeat(np.arange(S, dtype=np.int64), deg)
        eldst = (slot_ids % 128).astype(np.int32)
        etile = slot_ids // 128
        etab = np.searchsorted(tb[1:NTAB], esrc, side="right")
        key = etile * NTAB + etab
        o2 = np.argsort(key, kind="stable")
        esrc_l = (esrc[o2] - tb[etab[o2]]).astype(np.int16)
        eldst_s = eldst[o2]
        key_s = key[o2]
        kcnt = np.bincount(key_s, minlength=NT * NTAB).reshape(NT, NTAB)
        cnt_blocks[c] = (kcnt + 127) // 128
        per_core_raw.append((sn, deg, esrc_l, eldst_s, kcnt))

    nblk = cnt_blocks.max(axis=0)  # [NT, NTAB]
    nblk_t = nblk.sum(axis=1)
    NBLKSUM = int(nblk_t.sum())
    IDXW = 8 * NBLKSUM

    per_core = []
    for c in range(NCORES):
        sn, deg, esrc_l, eldst_s, kcnt = per_core_raw[c]
        kptr = np.zeros(NT * NTAB + 1, dtype=np.int64)
        np.cumsum(kcnt.reshape(-1), out=kptr[1:])
        idx2d = np.zeros((128, IDXW), dtype=np.int16)
        ldst2d = np.full((128, NBLKSUM), 200.0, dtype=np.float32)
        boff = 0
        for t in range(NT):
            for j in range(NTAB):
                nbj = int(nblk[t][j])
                if nbj == 0:
                    continue
                kk = t * NTAB + j
                cntk = int(kcnt[t][j])
                npad = nbj * 128
                sp = np.zeros(npad, dtype=np.int16)
                lp = np.full(npad, 200.0, dtype=np.float32)
                sp[:cntk] = esrc_l[kptr[kk]:kptr[kk + 1]]
                lp[:cntk] = eldst_s[kptr[kk]:kptr[kk + 1]]
                # idx wrapped in 16 partitions, replicated x8
                w16 = sp.reshape(npad // 16, 16).T  # [16, cols]
                for gidx in range(8):
                    idx2d[gidx * 16:(gidx + 1) * 16,
                          8 * boff: 8 * boff + npad // 16] = w16
                ldst2d[:, boff: boff + nbj] = lp.reshape(nbj, 128).T
                boff += nbj
        invdeg = (1.0 / np.maximum(deg, 1)).reshape(NT, 128).T.astype(np.float32)
        per_core.append({"idx2d": idx2d, "ldst2d": ldst2d, "invdeg": invdeg,
                         "slot_nodes": sn})

    meta = {
        "nblk": nblk.tolist(),
        "tsizes": [TBASE[j + 1] - TBASE[j] for j in range(NTAB)],
    }
    return per_core, meta


def kernel(**inputs):
    global LAST_RESULT, LAST_NC
    per_core, meta = _host_prep(inputs)
    nc = _build(meta)
    LAST_NC = nc

    feats16 = np.asarray(inputs["in_feats"]).astype(np.float16)
    shared = {}
    for j in range(NTAB):
        shared[f"feats{j}"] = np.ascontiguousarray(feats16[TBASE[j]:TBASE[j + 1]])

    def prep_sq(w):  # [512,512] -> [128, c*512+m] f16
        w = np.asarray(w, dtype=np.float32)
        return np.ascontiguousarray(
            w.reshape(4, 128, 512).transpose(1, 0, 2).reshape(128, 2048)
        ).astype(np.float16)

    def prep_gate(w):  # [1536,512] -> [128, c*1536 + j*128 + m] f16
        w = np.asarray(w, dtype=np.float32)
        a = w.reshape(12, 128, 4, 128).transpose(3, 2, 0, 1)  # [p, c, j, m]
        return np.ascontiguousarray(a.reshape(128, 6144)).astype(np.float16)

    def prep_bias(b, n):  # [n*128] -> [128, n] f32
        return np.ascontiguousarray(
            np.asarray(b, dtype=np.float32).reshape(n, 128).T)

    b_ih = np.asarray(inputs["b_ih"], dtype=np.float32)
    b_hh = np.asarray(inputs["b_hh"], dtype=np.float32)
    bsum = prep_bias(b_ih, 12).copy()
    bsum[:, 0:8] += prep_bias(b_hh, 12)[:, 0:8]
    bhn = np.repeat(b_hh[1024:].reshape(4, 128).T[:, :, None], 16,
                    axis=2).transpose(0, 1, 2)  # [p, c, 16]
    bhn = np.ascontiguousarray(bhn.reshape(128, 64)).astype(np.float16)

    shared["h_wconv"] = prep_sq(inputs["W_conv"])
    shared["h_wff1"] = prep_sq(inputs["W_ff1"])
    shared["h_wff2"] = prep_sq(inputs["W_ff2"])
    shared["h_wihT"] = prep_gate(inputs["W_ih"])
    shared["h_whhT"] = prep_gate(inputs["W_hh"])
    shared["h_bconv"] = prep_bias(inputs["b_conv"], 4)
    shared["h_bff1"] = prep_bias(inputs["b_ff1"], 4)
    shared["h_bff2"] = prep_bias(inputs["b_ff2"], 4)
    shared["h_bsum"] = np.ascontiguousarray(bsum)
    zbn = np.zeros((128, 128), np.float16)
    zbn[:, 64:128] = bhn
    shared["h_zbn"] = zbn

    in_maps = []
    for c in range(NCORES):
        m = dict(shared)
        m["idx2d"] = per_core[c]["idx2d"]
        m["ldst2d"] = per_core[c]["ldst2d"]
        m["h_invdeg"] = per_core[c]["invdeg"]
        in_maps.append(m)

    res = run_bass_kernel_spmd(nc, in_maps, list(range(NCORES)),
                               trace=bool(int(os.environ.get("KTRACE", "0"))))
    LAST_RESULT = res

    out_full = np.empty((N, D), dtype=np.float32)
    for c in range(NCORES):
        out_full[per_core[c]["slot_nodes"]] = res.results[c]["out"].astype(np.float32)
    return out_full
